# revision 62
# baseline (speedup 1.0000x reference)
"""Multi-head causal self-attention on 8 Trainium2 NeuronCores.

Problem: x[4,2048,1024] @ w_qkv[1024,3072] -> 16-head causal attention
         -> @ w_out[1024,1024] + b_out.

Sharding (hardcoded): 8 cores = 4 batches x 2 head-groups of 8 heads.
Core c handles batch b = c//2 and heads hg*8..hg*8+8, hg = c%2.
Each core computes a partial output [2048,1024] (its 8 heads pushed
through its w_out row-slice); host sums the two head-group partials per
batch and adds b_out.

Everything computes in fp16 (fp32 PSUM accumulation), which runs
matmuls at full 1 cycle/row PE rate and lands ~1e-3 relative error vs
the fp32 reference.

Device algorithm per core (all "transposed orientation" so the only
transpose needed -- x^T -- is done for free on the host):
  qT/kT [512, 2048] and v (natural [2048, 512]) via fp16 matmuls.
  Per head pair (2 heads = 128 partitions), per 512-wide query chunk:
    scores^T[j,i] for both heads into one 2-bank PSUM tile via
    row-tiled (K=64) matmul pairs, both narrowed to the causal range;
    ONE exp per key-tile on ScalarE (p^T fp16; for the first key tile
    the exp writes straight into the softmax-denominator accumulator
    pacc); causal masking via a 0/1 mask multiply restricted to the
    128-wide diagonal block; pacc += p^T on DVE (single wide add for
    tiles without a dead zone);
    out^T[d,i] += col-tiled matmuls into a zeroed PSUM bank (the
    zeroing matmul is load-bearing: a start=True clear does NOT reset
    the other col-group's accumulate state);
    denom rows via ones-vector matmuls (M=1) into shared zeroed denom
    banks (4 col-strip rows per bank).
  att^T (unnormalized) is copied to SBUF immediately (frees PSUM);
  denominator reciprocals are repacked through a DRAM bounce into a
  [128,16] tile so the (8-cycle/elem) DVE reciprocal runs over free
  size 16 instead of 512, then broadcast over partitions via a second
  DRAM bounce (fp16), then multiplied in-place into att^T.  The final
  group instead uses a zero-DMA path (wide fp16 reciprocal + K=1
  ones-matmul broadcast into the dead denom bank) to keep the 4-hop
  DRAM latency off the kernel tail.
  partial = att^T.T @ w_out_slice -> fp16 -> DMA to DRAM.

Emission order per 512-token stage s: QKV(s), attention(s),
out-projection(s-1).  The Tile scheduler prefers older (lower
priority-number) instructions among ready ones, so the attention
chain (QK matmul + exp, wrapped in tc.high_priority) runs at ScalarE
line rate while the out-projection and next stage's QKV matmuls fill
the PE's wait gaps.  QKV chains emit q0, k0, q1..q3, k1..k3, v: pair
0's attention starts after two chains, and later stages restart their
exp stream after a single q chain (old-chunk k/v are resident).
"""

import os
import sys

import numpy as np

if "/opt/trn_rl_repo" not in sys.path:
    sys.path.insert(0, "/opt/trn_rl_repo")

B, T, C = 4, 2048, 1024
H, D = 16, 64
NCORES = 8
HPC = 8  # heads per core
PAIRS = 4  # head pairs per core
CCH = 8  # contraction chunks over C (1024/128)
ICH = 4  # i (query) chunks of 512
NJT = 16  # j (key) tiles of 128

_CACHE = {}


def _build_program():
    import concourse.mybir as mybir
    import concourse.tile as tile
    from concourse import bacc

    f32 = mybir.dt.float32
    f16 = mybir.dt.float16
    EXP = mybir.ActivationFunctionType.Exp

    nc = bacc.Bacc(
        "TRN2", target_bir_lowering=False, debug=False, num_devices=NCORES
    )
    xt = nc.dram_tensor("xt", [C, T], f16, kind="ExternalInput").ap()
    wq = nc.dram_tensor("wq", [C, 512], f16, kind="ExternalInput").ap()
    wk = nc.dram_tensor("wk", [C, 512], f16, kind="ExternalInput").ap()
    wv = nc.dram_tensor("wv", [C, 512], f16, kind="ExternalInput").ap()
    wo = nc.dram_tensor("wo", [512, C], f16, kind="ExternalInput").ap()
    msk = nc.dram_tensor("msk", [128, 128], f16, kind="ExternalInput").ap()
    out = nc.dram_tensor("out", [T, C], f16, kind="ExternalOutput").ap()

    with tile.TileContext(nc) as tc:
        with (
            tc.tile_pool(name="wpool", bufs=1) as wpool,
            tc.tile_pool(name="wvpool", bufs=1) as wvpool,
            tc.tile_pool(name="wopool", bufs=1) as wopool,
            tc.tile_pool(name="xpool", bufs=1) as xpool,
            tc.tile_pool(name="qkpool", bufs=8) as qkpool,
            tc.tile_pool(name="vpool", bufs=16) as vpool,
            tc.tile_pool(name="apool", bufs=4) as apool,
            tc.tile_pool(name="ppool", bufs=18) as ppool,
            tc.tile_pool(name="cpool", bufs=1) as cpool,
            tc.tile_pool(name="rpool", bufs=4) as rpool,
            tc.tile_pool(name="qpool", bufs=6) as qpool,
            tc.tile_pool(name="opool", bufs=6) as opool,
            tc.tile_pool(name="dpool", bufs=8, space="DRAM") as dpool,
            tc.tile_pool(name="ps_a", bufs=2, space="PSUM") as ps_a,
            tc.tile_pool(name="ps_s", bufs=2, space="PSUM") as ps_s,
            tc.tile_pool(name="ps_o", bufs=1, space="PSUM") as ps_o,
            tc.tile_pool(name="ps_d", bufs=1, space="PSUM") as ps_d,
        ):
            # ---- constants / weights resident in SBUF ----
            mask_sb = cpool.tile([128, 128], f16, name="mask_sb")
            nc.sync.dma_start(out=mask_sb, in_=msk)
            ones_sb = cpool.tile([128, 1], f16, name="ones_sb")
            nc.vector.memset(ones_sb, 1.0)
            onesr_sb = cpool.tile([128, 64], f16, name="onesr_sb")
            nc.vector.memset(onesr_sb, 1.0)
            zer_sb = cpool.tile([128, 512], f16, name="zer_sb")
            nc.vector.memset(zer_sb, 0.0)
            # Prepay the ~2.7us exp table-set load during the DMA ramp.
            warm_sb = cpool.tile([1, 1], f16, name="warm_sb")
            nc.scalar.activation(warm_sb, ones_sb[0:1, :], EXP, scale=1.0)

            # Pack each weight into one wide SBUF tile loaded by a
            # single 1 MB DMA (chunk cc of weight W at cols
            # [512cc : 512cc+512]).  Few big DMAs beat many small ones:
            # the sync queue issues descriptors serially, and per-chunk
            # splits measured ~55us slower end-to-end.
            w_sb = {}

            def load_w(wname, wap, pool):
                t = pool.tile([128, 8 * 512], f16, name=wname, tag=wname)
                nc.sync.dma_start(
                    out=t.rearrange("p (cc c) -> p cc c", cc=8, c=512),
                    in_=wap.rearrange("(cc p) c -> p cc c", cc=8, p=128),
                )
                for cc in range(CCH):
                    w_sb[wname, cc] = t[:, cc * 512 : (cc + 1) * 512]

            # x^T packed as [128, 8*2048]: chunk cc at cols
            # [2048cc : 2048cc+2048]; loaded in 4 token-chunk DMAs of
            # 1 MB so stage-0 QKV starts early.
            xt_all = xpool.tile([128, 8 * T], f16, name="xt_all", tag="xt")
            xt_r = xt_all.rearrange("p (cc t) -> p cc t", cc=8, t=T)
            xt_src = xt.rearrange("(cc p) t -> p cc t", cc=8, p=128)
            nc.sync.dma_start(out=xt_r[:, :, 0:512], in_=xt_src[:, :, 0:512])
            load_w("wq", wq, wpool)
            load_w("wk", wk, wpool)
            load_w("wv", wv, wvpool)
            for t4 in range(1, 4):
                tsl = slice(t4 * 512, (t4 + 1) * 512)
                nc.sync.dma_start(out=xt_r[:, :, tsl], in_=xt_src[:, :, tsl])
            xt_sb = [
                xt_all[:, cc * T : (cc + 1) * T] for cc in range(CCH)
            ]
            # w_out packed the same way: [128, 4*1024], 1 DMA.
            wo_all = wopool.tile([128, 4 * C], f16, name="wo_all", tag="wo")
            nc.sync.dma_start(
                out=wo_all.rearrange("p (fc c) -> p fc c", fc=4, c=C),
                in_=wo.rearrange("(fc p) c -> p fc c", fc=4, p=128),
            )
            wo_sb = [wo_all[:, fc * C : (fc + 1) * C] for fc in range(4)]
            # PE HAM warmup on the (idle until b(0)) ps_o bank: ~4us of
            # throwaway matmuls during the DMA ramp so real chains run
            # at 2.4 GHz.
            wrm = ps_o.tile([128, 512], f32, name="wrm", tag="pso")
            for _ in range(20):
                nc.tensor.matmul(
                    wrm,
                    lhsT=zer_sb[:, 0:128],
                    rhs=zer_sb,
                    start=True,
                    stop=True,
                    skip_group_check=True,
                )

            # ---- persistent activations ----
            qT = [
                qkpool.tile([128, T], f16, name=f"qT_{p}", tag="qk")
                for p in range(PAIRS)
            ]
            kT = [
                qkpool.tile([128, T], f16, name=f"kT_{p}", tag="qk")
                for p in range(PAIRS)
            ]
            v_sb = [
                vpool.tile([128, 512], f16, name=f"v_{j}", tag="v")
                for j in range(NJT)
            ]
            att = [
                apool.tile([128, T], f16, name=f"att_{p}", tag="att")
                for p in range(PAIRS)
            ]

            def phase_a(t4):
                """QKV projections for token chunk t4 (512 tokens)."""
                tsl4 = slice(t4 * 512, (t4 + 1) * 512)
                xts = [xt_sb[cc][:, tsl4] for cc in range(CCH)]
                # Chain order q0, k0, q1..q3, k1..k3: pair 0's QK->exp
                # needs only q0+k0 (first exp of stage 0 ~13us), and
                # later stages' attention restarts after just the q of
                # the new chunk (old-chunk k/v are resident).
                for wname, n in (
                    [("wq", 0), ("wk", 0)]
                    + [("wq", n) for n in range(1, PAIRS)]
                    + [("wk", n) for n in range(1, PAIRS)]
                ):
                    dst = qT if wname == "wq" else kT
                    ps = ps_a.tile([128, 512], f32, name="ps_qk", tag="psA")
                    for cc in range(CCH):
                        nc.tensor.matmul(
                            ps,
                            lhsT=w_sb[wname, cc][:, n * 128 : (n + 1) * 128],
                            rhs=xts[cc][:],
                            start=(cc == 0),
                            stop=(cc == CCH - 1),
                        )
                    if wname == "wq" and n == 0:
                        # Pair 0's q evacuation unblocks the next
                        # stage's whole exp stream; at normal priority
                        # the compile-time schedule places it (and
                        # therefore the first QK) ~10us late behind
                        # the previous stage's DVE backlog.
                        with tc.high_priority():
                            nc.vector.tensor_copy(
                                dst[n][:, t4 * 512 : (t4 + 1) * 512], ps
                            )
                    else:
                        nc.vector.tensor_copy(
                            dst[n][:, t4 * 512 : (t4 + 1) * 512], ps
                        )
                for tt in range(4):
                    ps = ps_a.tile([128, 512], f32, name="ps_v", tag="psA")
                    for cc in range(CCH):
                        nc.tensor.matmul(
                            ps,
                            lhsT=xts[cc][:, tt * 128 : (tt + 1) * 128],
                            rhs=w_sb["wv", cc][:],
                            start=(cc == 0),
                            stop=(cc == CCH - 1),
                        )
                    nc.vector.tensor_copy(v_sb[t4 * 4 + tt], ps)

            def phase_b(ic):
                """Attention for query chunk ic (512 queries), all pairs."""
                isl = slice(ic * 512, (ic + 1) * 512)
                njt = 4 * ic + 4
                # Two denominator banks per ic: bank A rows {0,32,64,96}
                # = pairs 0,1; bank B = pairs 2,3.  One zeroing matmul
                # each establishes the group and write-ordering.
                dbanks = []
                for g in range(2):
                    bank = ps_d.tile([128, 512], f32, name=f"ps_den{g}", tag="psd")
                    nc.tensor.matmul(
                        bank,
                        lhsT=zer_sb[:, 0:128],
                        rhs=zer_sb,
                        start=True,
                        stop=False,
                        skip_group_check=True,
                    )
                    dbanks.append(bank)

                def norm_group(g):
                    """1/denominators for pairs 2g, 2g+1 -> rdb + in-place mul.

                    [4,512] -> [128,16] DRAM repack so the 8-cyc/elem
                    reciprocal runs over free size 16; partition
                    broadcast via a second DRAM bounce.  The final
                    group routes its DMAs through the otherwise-idle
                    scalar HWDGE queue (exp is done by then) to keep
                    the kernel tail off the busy sync queue."""
                    bank = dbanks[g]
                    eng = nc.sync
                    rec = rpool.tile([128, 512], f32, name="rec", tag="rec")
                    nc.vector.tensor_copy(rec[0:97, :], bank[0:97, :])
                    if ic == 3 and g == 1:
                        # Final group: low-latency path (the 4-hop DRAM
                        # bounce would sit on the kernel tail).  Wide
                        # fp16 reciprocal, then broadcast 1/D over
                        # partitions with K=1 ones-matmuls into the
                        # now-dead denominator bank.
                        reci = rpool.tile(
                            [128, 512], f16, name="reci", tag="reci"
                        )
                        with nc.allow_low_precision(
                            reason="1/denom in fp16; denom ~1e3, fine"
                        ):
                            nc.vector.reciprocal(reci[0:97, :], rec[0:97, :])
                        for lp in range(2):
                            pr = 2 * g + lp
                            for hh in range(2):
                                r = 32 * (2 * lp + hh)
                                nc.tensor.matmul(
                                    bank[64 * hh : 64 * hh + 64, :],
                                    lhsT=onesr_sb[r : r + 1, :],
                                    rhs=reci[r : r + 1, :],
                                    start=True,
                                    stop=True,
                                    tile_position=(r, 64 * hh),
                                    skip_group_check=True,
                                )
                            asl = att[pr][:, isl]
                            nc.vector.tensor_mul(asl, asl, bank)
                        return
                    dsc = dpool.tile([4, 512], f32, name="dsc", tag="dsc")
                    eng.dma_start(
                        out=dsc.unsqueeze(1),
                        in_=rec.rearrange("(a b) c -> a b c", a=4, b=32)[
                            :, 0:1, :
                        ],
                    )
                    rdr = rpool.tile([128, 16], f32, name="rdr", tag="rdr")
                    eng.dma_start(
                        out=rdr,
                        in_=dsc.rearrange("a (b c) -> (a b) c", b=32, c=16),
                    )
                    rdi = rpool.tile([128, 16], f16, name="rdi", tag="rdi")
                    with nc.allow_low_precision(
                        reason="1/denom broadcast in fp16; denom ~1e3, fine"
                    ):
                        nc.vector.reciprocal(rdi, rdr)
                    dsc2 = dpool.tile([128, 16], f16, name="dsc2", tag="dsc2")
                    eng.dma_start(out=dsc2, in_=rdi)
                    dsc2r = dsc2.rearrange("(a b) c -> a (b c)", a=4, b=32)
                    for lp in range(2):
                        pr = 2 * g + lp
                        rdb = rpool.tile([128, 512], f16, name="rdb", tag="rdb")
                        eng.dma_start(
                            out=rdb[0:64, :],
                            in_=dsc2r[2 * lp : 2 * lp + 1, :].broadcast_to(
                                [64, 512]
                            ),
                        )
                        eng.dma_start(
                            out=rdb[64:128, :],
                            in_=dsc2r[2 * lp + 1 : 2 * lp + 2, :].broadcast_to(
                                [64, 512]
                            ),
                        )
                        asl = att[pr][:, isl]
                        nc.vector.tensor_mul(asl, asl, rdb)

                for pr in range(PAIRS):
                    ps_out = ps_o.tile([128, 512], f32, name="ps_out", tag="pso")
                    dbank = dbanks[pr // 2]
                    dp0 = 64 * (pr % 2)
                    dp1 = 64 * (pr % 2) + 32
                    # Zero the whole ps_out bank in one matmul:
                    # establishes the accumulation group and a WAW dep
                    # ordering it before both col-tiled sub-chains.
                    nc.tensor.matmul(
                        ps_out,
                        lhsT=zer_sb[:, 0:128],
                        rhs=zer_sb,
                        start=True,
                        stop=False,
                        skip_group_check=True,
                    )
                    pacc = qpool.tile([128, 1024], f16, name="pacc", tag="pacc")
                    pTs = {}

                    def emit_qk_exp(jt):
                        jsl = slice(jt * 128, (jt + 1) * 128)
                        dpos = jt - 4 * ic
                        # Causal: query columns below 128*dpos within
                        # this chunk see none of this key tile; both QK
                        # matmuls narrow to [ioff:].
                        ioff = 128 * dpos if dpos > 0 else 0
                        islq = slice(ic * 512 + ioff, (ic + 1) * 512)
                        sb = ps_s.tile([128, 1024], f32, name="sb", tag="pss")
                        # The QK -> exp chain runs at high priority so
                        # the PE feeds ScalarE (the softmax engine) as
                        # soon as inputs land; PV and projection
                        # matmuls fill the PE's remaining capacity.
                        with tc.high_priority():
                            nc.tensor.matmul(
                                sb[:, ioff:512],
                                lhsT=kT[pr][0:64, jsl],
                                rhs=qT[pr][0:64, islq],
                                start=True,
                                stop=True,
                                tile_position=(0, 0),
                            )
                            nc.tensor.matmul(
                                sb[:, 512 + ioff : 1024],
                                lhsT=kT[pr][64:128, jsl],
                                rhs=qT[pr][64:128, islq],
                                start=True,
                                stop=True,
                                tile_position=(64, 0),
                            )
                            # One exp covers both heads; the dead zone
                            # between the halves on diagonal tiles
                            # holds stale-but-finite scores and is
                            # never read downstream.  The first key
                            # tile's exp writes straight into the pacc
                            # accumulator.
                            if jt == 0:
                                pTb = pacc
                                nc.scalar.activation(
                                    pacc[:, 0:1024], sb[:, 0:1024], EXP,
                                    scale=0.125,
                                )
                            else:
                                pTb = ppool.tile(
                                    [128, 1024], f16, name="pTb", tag="pT"
                                )
                                nc.scalar.activation(
                                    pTb[:, ioff:1024], sb[:, ioff:1024], EXP,
                                    scale=0.125,
                                )
                        pTs[jt] = pTb
                        if dpos >= 0:
                            # Mask only the 128-wide diagonal block;
                            # everything past it is fully visible.
                            msl = mask_sb[:, :]
                            nc.vector.tensor_mul(
                                pTb[:, ioff : ioff + 128],
                                pTb[:, ioff : ioff + 128],
                                msl,
                            )
                            nc.vector.tensor_mul(
                                pTb[:, 512 + ioff : 512 + ioff + 128],
                                pTb[:, 512 + ioff : 512 + ioff + 128],
                                msl,
                            )

                    def emit_pv(jt):
                        dpos = jt - 4 * ic
                        ioff = 128 * dpos if dpos > 0 else 0
                        pTb = pTs[jt]
                        pT0 = pTb[:, 0:512]
                        pT1 = pTb[:, 512:1024]
                        last = jt == njt - 1
                        vt = v_sb[jt]
                        nc.tensor.matmul(
                            ps_out[0:64, ioff:512],
                            lhsT=vt[:, pr * 128 : pr * 128 + 64],
                            rhs=pT0[:, ioff:512],
                            start=False,
                            stop=False,
                            tile_position=(0, 0),
                            skip_group_check=True,
                        )
                        nc.tensor.matmul(
                            ps_out[64:128, ioff:512],
                            lhsT=vt[:, pr * 128 + 64 : pr * 128 + 128],
                            rhs=pT1[:, ioff:512],
                            start=False,
                            stop=last,
                            tile_position=(0, 64),
                            skip_group_check=True,
                        )
                        if jt > 0:
                            if ioff == 0:
                                nc.vector.tensor_add(
                                    pacc[:, 0:1024], pacc[:, 0:1024],
                                    pTb[:, 0:1024],
                                )
                            else:
                                nc.vector.tensor_add(
                                    pacc[:, ioff:512],
                                    pacc[:, ioff:512],
                                    pT0[:, ioff:512],
                                )
                                nc.vector.tensor_add(
                                    pacc[:, 512 + ioff : 1024],
                                    pacc[:, 512 + ioff : 1024],
                                    pT1[:, ioff:512],
                                )

                    # PV lags QK/exp by LAG key tiles: its inputs are
                    # always long-ready, so PE drains overlap instead
                    # of waiting on just-finished exps, and the tail
                    # PVs of this pair overlap the next pair's QK/exp
                    # ramp.
                    LAG = 2
                    for jt in range(njt):
                        emit_qk_exp(jt)
                        if jt >= LAG:
                            emit_pv(jt - LAG)
                    for jt in range(max(0, njt - LAG), njt):
                        emit_pv(jt)
                    # Partition-reduce the accumulated p-sums into the
                    # shared denominator bank.  High priority so the
                    # compile-time PE order keeps the two col-split
                    # M=1 matmuls adjacent -- adjacent issue makes them
                    # run concurrently (disjoint col groups), halving
                    # their stream time.
                    with tc.high_priority():
                        nc.tensor.matmul(
                            dbank[dp0 : dp0 + 1, :],
                            lhsT=ones_sb,
                            rhs=pacc[:, 0:512],
                            start=False,
                            stop=False,
                            tile_position=(0, dp0),
                            skip_group_check=True,
                        )
                        nc.tensor.matmul(
                            dbank[dp1 : dp1 + 1, :],
                            lhsT=ones_sb,
                            rhs=pacc[:, 512:1024],
                            start=False,
                            stop=False,
                            tile_position=(0, dp1),
                            skip_group_check=True,
                        )
                    # Unnormalized copy frees ps_out quickly (high
                    # priority: it gates the next pair's accumulation
                    # group); normalization happens in-place on att
                    # once the broadcast lands.
                    asl = att[pr][:, isl]
                    with tc.high_priority():
                        nc.vector.tensor_copy(asl, ps_out)
                    if pr % 2 == 1:
                        norm_group(pr // 2)

            def phase_c(s):
                """Output projection for token tiles 4s..4s+4."""
                for tt in range(4 * s, 4 * s + 4):
                    tsl = slice(tt * 128, (tt + 1) * 128)
                    ost = opool.tile([128, 1024], f16, name="ost", tag="ost")
                    for n in range(2):
                        ps = ps_a.tile([128, 512], f32, name="ps_c", tag="psA")
                        for fc in range(4):
                            nc.tensor.matmul(
                                ps,
                                lhsT=att[fc][:, tsl],
                                rhs=wo_sb[fc][:, n * 512 : (n + 1) * 512],
                                start=(fc == 0),
                                stop=(fc == 3),
                            )
                        # ScalarE copy: exp is idle exactly when these
                        # evacuations bunch (stage boundaries), and it
                        # keeps DVE free for the q-chain evacuations
                        # that unblock the next exp stream.
                        nc.scalar.copy(ost[:, n * 512 : (n + 1) * 512], ps)
                    nc.sync.dma_start(out=out[tsl, :], in_=ost)

            for s in range(4):
                phase_a(s)
                if s >= 1:
                    phase_c(s - 1)
                phase_b(s)
            phase_c(3)

    nc.compile()
    return nc


def _get_program():
    if "nc" not in _CACHE:
        _CACHE["nc"] = _build_program()
    return _CACHE["nc"]


def _make_mask():
    # msk[jj, c] = 1 if c >= jj else 0: the 128-wide diagonal block.
    jj = np.arange(128)[:, None]
    c = np.arange(128)[None, :]
    return (c >= jj).astype(np.float16)


def _make_in_maps(x, w_qkv, w_out):
    mask = _make_mask()
    in_maps = []
    for core in range(NCORES):
        b, hg = core // 2, core % 2
        cs = slice(hg * 512, (hg + 1) * 512)
        f16 = np.float16
        in_maps.append(
            {
                "xt": np.ascontiguousarray(x[b].T).astype(f16),
                "wq": np.ascontiguousarray(
                    w_qkv[:, hg * 512 : hg * 512 + 512]
                ).astype(f16),
                "wk": np.ascontiguousarray(
                    w_qkv[:, 1024 + hg * 512 : 1024 + hg * 512 + 512]
                ).astype(f16),
                "wv": np.ascontiguousarray(
                    w_qkv[:, 2048 + hg * 512 : 2048 + hg * 512 + 512]
                ).astype(f16),
                "wo": np.ascontiguousarray(w_out[cs, :]).astype(f16),
                "msk": mask,
            }
        )
    return in_maps


def _run_device(in_maps, trace=False):
    from concourse.bass_utils import run_bass_kernel_spmd

    nc = _get_program()
    return run_bass_kernel_spmd(
        nc, in_maps, core_ids=list(range(NCORES)), trace=trace
    )


def kernel(x, w_qkv, w_out, b_out):
    x = np.asarray(x, dtype=np.float32)
    w_qkv = np.asarray(w_qkv, dtype=np.float32)
    w_out = np.asarray(w_out, dtype=np.float32)
    b_out = np.asarray(b_out, dtype=np.float32)

    res = _run_device(_make_in_maps(x, w_qkv, w_out)).results
    out = np.empty((B, T, C), dtype=np.float32)
    for b in range(B):
        out[b] = (
            res[2 * b]["out"].astype(np.float32)
            + res[2 * b + 1]["out"].astype(np.float32)
            + b_out
        )
    return out


# revision 63
# speedup vs baseline: 1.0409x; 1.0409x over previous
"""Multi-head causal self-attention on 8 Trainium2 NeuronCores.

Problem: x[4,2048,1024] @ w_qkv[1024,3072] -> 16-head causal attention
         -> @ w_out[1024,1024] + b_out.

Sharding (hardcoded): 8 cores = 4 batches x 2 head-groups of 8 heads.
Core c handles batch b = c//2 and heads hg*8..hg*8+8, hg = c%2.
Each core computes a partial output [2048,1024] (its 8 heads pushed
through its w_out row-slice); host sums the two head-group partials per
batch and adds b_out.

Everything computes in fp16 (fp32 PSUM accumulation), which runs
matmuls at full 1 cycle/row PE rate and lands ~1e-3 relative error vs
the fp32 reference.

Device algorithm per core (all "transposed orientation" so the only
transpose needed -- x^T -- is done for free on the host):
  qT/kT [512, 2048] and v (natural [2048, 512]) via fp16 matmuls.
  Per head pair (2 heads = 128 partitions), per 512-wide query chunk:
    scores^T[j,i] for both heads into one 2-bank PSUM tile via
    row-tiled (K=64) matmul pairs, both narrowed to the causal range;
    ONE exp per key-tile on ScalarE (p^T fp16; for the first key tile
    the exp writes straight into the softmax-denominator accumulator
    pacc); causal masking via a 0/1 mask multiply restricted to the
    128-wide diagonal block; pacc += p^T on DVE (single wide add for
    tiles without a dead zone);
    out^T[d,i] += col-tiled matmuls into a zeroed PSUM bank (the
    zeroing matmul is load-bearing: a start=True clear does NOT reset
    the other col-group's accumulate state);
    denom rows via ones-vector matmuls (M=1) into shared zeroed denom
    banks (4 col-strip rows per bank).
  att^T (unnormalized) is copied to SBUF immediately (frees PSUM);
  denominator reciprocals are repacked through a DRAM bounce into a
  [128,16] tile so the (8-cycle/elem) DVE reciprocal runs over free
  size 16 instead of 512, then broadcast over partitions via a second
  DRAM bounce (fp16), then multiplied in-place into att^T.  The final
  group instead uses a zero-DMA path (wide fp16 reciprocal + K=1
  ones-matmul broadcast into the dead denom bank) to keep the 4-hop
  DRAM latency off the kernel tail.
  partial = att^T.T @ w_out_slice -> fp16 -> DMA to DRAM.

Emission order per 512-token stage s: QKV(s), attention(s),
out-projection(s-1).  The Tile scheduler prefers older (lower
priority-number) instructions among ready ones, so the attention
chain (QK matmul + exp, wrapped in tc.high_priority) runs at ScalarE
line rate while the out-projection and next stage's QKV matmuls fill
the PE's wait gaps.  QKV chains emit q0, k0, q1..q3, k1..k3, v: pair
0's attention starts after two chains, and later stages restart their
exp stream after a single q chain (old-chunk k/v are resident).
"""

import os
import sys

import numpy as np

if "/opt/trn_rl_repo" not in sys.path:
    sys.path.insert(0, "/opt/trn_rl_repo")

B, T, C = 4, 2048, 1024
H, D = 16, 64
NCORES = 8
HPC = 8  # heads per core
PAIRS = 4  # head pairs per core
CCH = 8  # contraction chunks over C (1024/128)
ICH = 4  # i (query) chunks of 512
NJT = 16  # j (key) tiles of 128

_CACHE = {}


def _build_program():
    import concourse.mybir as mybir
    import concourse.tile as tile
    from concourse import bacc

    f32 = mybir.dt.float32
    f16 = mybir.dt.float16
    EXP = mybir.ActivationFunctionType.Exp

    nc = bacc.Bacc(
        "TRN2", target_bir_lowering=False, debug=False, num_devices=NCORES
    )
    xt = nc.dram_tensor("xt", [C, T], f16, kind="ExternalInput").ap()
    wq = nc.dram_tensor("wq", [C, 512], f16, kind="ExternalInput").ap()
    wk = nc.dram_tensor("wk", [C, 512], f16, kind="ExternalInput").ap()
    wv = nc.dram_tensor("wv", [C, 512], f16, kind="ExternalInput").ap()
    wo = nc.dram_tensor("wo", [512, C], f16, kind="ExternalInput").ap()
    msk = nc.dram_tensor("msk", [128, 128], f16, kind="ExternalInput").ap()
    out = nc.dram_tensor("out", [T, C], f16, kind="ExternalOutput").ap()

    with tile.TileContext(nc) as tc:
        with (
            tc.tile_pool(name="wpool", bufs=1) as wpool,
            tc.tile_pool(name="wvpool", bufs=1) as wvpool,
            tc.tile_pool(name="wopool", bufs=1) as wopool,
            tc.tile_pool(name="xpool", bufs=1) as xpool,
            tc.tile_pool(name="qkpool", bufs=8) as qkpool,
            tc.tile_pool(name="vpool", bufs=16) as vpool,
            tc.tile_pool(name="apool", bufs=4) as apool,
            tc.tile_pool(name="ppool", bufs=18) as ppool,
            tc.tile_pool(name="cpool", bufs=1) as cpool,
            tc.tile_pool(name="rpool", bufs=4) as rpool,
            tc.tile_pool(name="qpool", bufs=6) as qpool,
            tc.tile_pool(name="opool", bufs=6) as opool,
            tc.tile_pool(name="dpool", bufs=8, space="DRAM") as dpool,
            tc.tile_pool(name="ps_a", bufs=2, space="PSUM") as ps_a,
            tc.tile_pool(name="ps_s", bufs=2, space="PSUM") as ps_s,
            tc.tile_pool(name="ps_o", bufs=1, space="PSUM") as ps_o,
            tc.tile_pool(name="ps_d", bufs=1, space="PSUM") as ps_d,
        ):
            # ---- constants / weights resident in SBUF ----
            mask_sb = cpool.tile([128, 128], f16, name="mask_sb")
            nc.sync.dma_start(out=mask_sb, in_=msk)
            ones_sb = cpool.tile([128, 1], f16, name="ones_sb")
            nc.vector.memset(ones_sb, 1.0)
            onesr_sb = cpool.tile([128, 64], f16, name="onesr_sb")
            nc.vector.memset(onesr_sb, 1.0)
            zer_sb = cpool.tile([128, 512], f16, name="zer_sb")
            nc.vector.memset(zer_sb, 0.0)
            # Prepay the ~2.7us exp table-set load during the DMA ramp.
            warm_sb = cpool.tile([1, 1], f16, name="warm_sb")
            nc.scalar.activation(warm_sb, ones_sb[0:1, :], EXP, scale=1.0)

            # Pack each weight into one wide SBUF tile loaded by a
            # single 1 MB DMA (chunk cc of weight W at cols
            # [512cc : 512cc+512]).  Few big DMAs beat many small ones:
            # the sync queue issues descriptors serially, and per-chunk
            # splits measured ~55us slower end-to-end.
            w_sb = {}

            def load_w(wname, wap, pool):
                t = pool.tile([128, 8 * 512], f16, name=wname, tag=wname)
                nc.sync.dma_start(
                    out=t.rearrange("p (cc c) -> p cc c", cc=8, c=512),
                    in_=wap.rearrange("(cc p) c -> p cc c", cc=8, p=128),
                )
                for cc in range(CCH):
                    w_sb[wname, cc] = t[:, cc * 512 : (cc + 1) * 512]

            # x^T packed as [128, 8*2048]: chunk cc at cols
            # [2048cc : 2048cc+2048]; loaded in 4 token-chunk DMAs of
            # 1 MB so stage-0 QKV starts early.
            xt_all = xpool.tile([128, 8 * T], f16, name="xt_all", tag="xt")
            xt_r = xt_all.rearrange("p (cc t) -> p cc t", cc=8, t=T)
            xt_src = xt.rearrange("(cc p) t -> p cc t", cc=8, p=128)
            nc.sync.dma_start(out=xt_r[:, :, 0:512], in_=xt_src[:, :, 0:512])
            load_w("wq", wq, wpool)
            load_w("wk", wk, wpool)
            load_w("wv", wv, wvpool)
            for t4 in range(1, 4):
                tsl = slice(t4 * 512, (t4 + 1) * 512)
                nc.sync.dma_start(out=xt_r[:, :, tsl], in_=xt_src[:, :, tsl])
            xt_sb = [
                xt_all[:, cc * T : (cc + 1) * T] for cc in range(CCH)
            ]
            # w_out packed the same way: [128, 4*1024], 1 DMA.
            wo_all = wopool.tile([128, 4 * C], f16, name="wo_all", tag="wo")
            nc.sync.dma_start(
                out=wo_all.rearrange("p (fc c) -> p fc c", fc=4, c=C),
                in_=wo.rearrange("(fc p) c -> p fc c", fc=4, p=128),
            )
            wo_sb = [wo_all[:, fc * C : (fc + 1) * C] for fc in range(4)]
            # PE HAM warmup on the (idle until b(0)) ps_o bank: ~4us of
            # throwaway matmuls during the DMA ramp so real chains run
            # at 2.4 GHz.
            wrm = ps_o.tile([128, 512], f32, name="wrm", tag="pso")
            for _ in range(20):
                nc.tensor.matmul(
                    wrm,
                    lhsT=zer_sb[:, 0:128],
                    rhs=zer_sb,
                    start=True,
                    stop=True,
                    skip_group_check=True,
                )

            # ---- persistent activations ----
            qT = [
                qkpool.tile([128, T], f16, name=f"qT_{p}", tag="qk")
                for p in range(PAIRS)
            ]
            kT = [
                qkpool.tile([128, T], f16, name=f"kT_{p}", tag="qk")
                for p in range(PAIRS)
            ]
            v_sb = [
                vpool.tile([128, 512], f16, name=f"v_{j}", tag="v")
                for j in range(NJT)
            ]
            att = [
                apool.tile([128, T], f16, name=f"att_{p}", tag="att")
                for p in range(PAIRS)
            ]

            def phase_a(t4):
                """QKV projections for token chunk t4 (512 tokens)."""
                tsl4 = slice(t4 * 512, (t4 + 1) * 512)
                xts = [xt_sb[cc][:, tsl4] for cc in range(CCH)]
                # Chain order q0, k0, q1..q3, k1..k3: pair 0's QK->exp
                # needs only q0+k0 (first exp of stage 0 ~13us), and
                # later stages' attention restarts after just the q of
                # the new chunk (old-chunk k/v are resident).
                for wname, n in (
                    [("wq", 0), ("wk", 0)]
                    + [("wq", n) for n in range(1, PAIRS)]
                    + [("wk", n) for n in range(1, PAIRS)]
                ):
                    dst = qT if wname == "wq" else kT
                    ps = ps_a.tile([128, 512], f32, name="ps_qk", tag="psA")
                    for cc in range(CCH):
                        nc.tensor.matmul(
                            ps,
                            lhsT=w_sb[wname, cc][:, n * 128 : (n + 1) * 128],
                            rhs=xts[cc][:],
                            start=(cc == 0),
                            stop=(cc == CCH - 1),
                        )
                    if wname == "wq" and n == 0:
                        # Pair 0's q evacuation unblocks the next
                        # stage's whole exp stream; at normal priority
                        # the compile-time schedule places it (and
                        # therefore the first QK) ~10us late behind
                        # the previous stage's DVE backlog.
                        with tc.high_priority():
                            nc.vector.tensor_copy(
                                dst[n][:, t4 * 512 : (t4 + 1) * 512], ps
                            )
                    else:
                        nc.vector.tensor_copy(
                            dst[n][:, t4 * 512 : (t4 + 1) * 512], ps
                        )
                for tt in range(4):
                    ps = ps_a.tile([128, 512], f32, name="ps_v", tag="psA")
                    for cc in range(CCH):
                        nc.tensor.matmul(
                            ps,
                            lhsT=xts[cc][:, tt * 128 : (tt + 1) * 128],
                            rhs=w_sb["wv", cc][:],
                            start=(cc == 0),
                            stop=(cc == CCH - 1),
                        )
                    nc.vector.tensor_copy(v_sb[t4 * 4 + tt], ps)

            def phase_b(ic):
                """Attention for query chunk ic (512 queries), all pairs."""
                isl = slice(ic * 512, (ic + 1) * 512)
                njt = 4 * ic + 4
                # Two denominator banks per ic: bank A rows {0,32,64,96}
                # = pairs 0,1; bank B = pairs 2,3.  One zeroing matmul
                # each establishes the group and write-ordering.
                dbanks = []
                for g in range(2):
                    bank = ps_d.tile([128, 512], f32, name=f"ps_den{g}", tag="psd")
                    nc.tensor.matmul(
                        bank,
                        lhsT=zer_sb[:, 0:128],
                        rhs=zer_sb,
                        start=True,
                        stop=False,
                        skip_group_check=True,
                    )
                    dbanks.append(bank)

                def norm_group(g):
                    """1/denominators for pairs 2g, 2g+1 -> rdb + in-place mul.

                    [4,512] -> [128,16] DRAM repack so the 8-cyc/elem
                    reciprocal runs over free size 16; partition
                    broadcast via a second DRAM bounce.  The final
                    group routes its DMAs through the otherwise-idle
                    scalar HWDGE queue (exp is done by then) to keep
                    the kernel tail off the busy sync queue."""
                    bank = dbanks[g]
                    eng = nc.sync
                    rec = rpool.tile([128, 512], f32, name="rec", tag="rec")
                    nc.vector.tensor_copy(rec[0:97, :], bank[0:97, :])
                    if ic == 3 and g == 1:
                        # Final group: low-latency path (the 4-hop DRAM
                        # bounce would sit on the kernel tail).  Wide
                        # fp16 reciprocal, then broadcast 1/D over
                        # partitions with K=1 ones-matmuls into the
                        # now-dead denominator bank.
                        reci = rpool.tile(
                            [128, 512], f16, name="reci", tag="reci"
                        )
                        with nc.allow_low_precision(
                            reason="1/denom in fp16; denom ~1e3, fine"
                        ):
                            nc.vector.reciprocal(reci[0:97, :], rec[0:97, :])
                        for lp in range(2):
                            pr = 2 * g + lp
                            for hh in range(2):
                                r = 32 * (2 * lp + hh)
                                nc.tensor.matmul(
                                    bank[64 * hh : 64 * hh + 64, :],
                                    lhsT=onesr_sb[r : r + 1, :],
                                    rhs=reci[r : r + 1, :],
                                    start=True,
                                    stop=True,
                                    tile_position=(r, 64 * hh),
                                    skip_group_check=True,
                                )
                            asl = att[pr][:, isl]
                            nc.vector.tensor_mul(asl, asl, bank)
                        return
                    dsc = dpool.tile([4, 512], f32, name="dsc", tag="dsc")
                    eng.dma_start(
                        out=dsc.unsqueeze(1),
                        in_=rec.rearrange("(a b) c -> a b c", a=4, b=32)[
                            :, 0:1, :
                        ],
                    )
                    rdr = rpool.tile([128, 16], f32, name="rdr", tag="rdr")
                    eng.dma_start(
                        out=rdr,
                        in_=dsc.rearrange("a (b c) -> (a b) c", b=32, c=16),
                    )
                    rdi = rpool.tile([128, 16], f16, name="rdi", tag="rdi")
                    with nc.allow_low_precision(
                        reason="1/denom broadcast in fp16; denom ~1e3, fine"
                    ):
                        nc.vector.reciprocal(rdi, rdr)
                    dsc2 = dpool.tile([128, 16], f16, name="dsc2", tag="dsc2")
                    eng.dma_start(out=dsc2, in_=rdi)
                    dsc2r = dsc2.rearrange("(a b) c -> a (b c)", a=4, b=32)
                    for lp in range(2):
                        pr = 2 * g + lp
                        rdb = rpool.tile([128, 512], f16, name="rdb", tag="rdb")
                        eng.dma_start(
                            out=rdb[0:64, :],
                            in_=dsc2r[2 * lp : 2 * lp + 1, :].broadcast_to(
                                [64, 512]
                            ),
                        )
                        eng.dma_start(
                            out=rdb[64:128, :],
                            in_=dsc2r[2 * lp + 1 : 2 * lp + 2, :].broadcast_to(
                                [64, 512]
                            ),
                        )
                        asl = att[pr][:, isl]
                        nc.vector.tensor_mul(asl, asl, rdb)

                for pr in range(PAIRS):
                    ps_out = ps_o.tile([128, 512], f32, name="ps_out", tag="pso")
                    dbank = dbanks[pr // 2]
                    dp0 = 64 * (pr % 2)
                    dp1 = 64 * (pr % 2) + 32
                    # Zero the whole ps_out bank in one matmul:
                    # establishes the accumulation group and a WAW dep
                    # ordering it before both col-tiled sub-chains.
                    nc.tensor.matmul(
                        ps_out,
                        lhsT=zer_sb[:, 0:128],
                        rhs=zer_sb,
                        start=True,
                        stop=False,
                        skip_group_check=True,
                    )
                    pacc = qpool.tile([128, 1024], f16, name="pacc", tag="pacc")
                    pTs = {}

                    def emit_qk_exp(jt):
                        jsl = slice(jt * 128, (jt + 1) * 128)
                        dpos = jt - 4 * ic
                        # Causal: query columns below 128*dpos within
                        # this chunk see none of this key tile; both QK
                        # matmuls narrow to [ioff:].
                        ioff = 128 * dpos if dpos > 0 else 0
                        islq = slice(ic * 512 + ioff, (ic + 1) * 512)
                        sb = ps_s.tile([128, 1024], f32, name="sb", tag="pss")
                        # The QK -> exp chain runs at high priority so
                        # the PE feeds ScalarE (the softmax engine) as
                        # soon as inputs land; PV and projection
                        # matmuls fill the PE's remaining capacity.
                        with tc.high_priority():
                            nc.tensor.matmul(
                                sb[:, ioff:512],
                                lhsT=kT[pr][0:64, jsl],
                                rhs=qT[pr][0:64, islq],
                                start=True,
                                stop=True,
                                tile_position=(0, 0),
                            )
                            nc.tensor.matmul(
                                sb[:, 512 + ioff : 1024],
                                lhsT=kT[pr][64:128, jsl],
                                rhs=qT[pr][64:128, islq],
                                start=True,
                                stop=True,
                                tile_position=(64, 0),
                            )
                            # One exp covers both heads; the dead zone
                            # between the halves on diagonal tiles
                            # holds stale-but-finite scores and is
                            # never read downstream.  The first key
                            # tile's exp writes straight into the pacc
                            # accumulator.
                            if jt == 0:
                                pTb = pacc
                                nc.scalar.activation(
                                    pacc[:, 0:1024], sb[:, 0:1024], EXP,
                                    scale=0.125,
                                )
                            else:
                                pTb = ppool.tile(
                                    [128, 1024], f16, name="pTb", tag="pT"
                                )
                                nc.scalar.activation(
                                    pTb[:, ioff:1024], sb[:, ioff:1024], EXP,
                                    scale=0.125,
                                )
                        pTs[jt] = pTb
                        if dpos >= 0:
                            # Mask only the 128-wide diagonal block;
                            # everything past it is fully visible.
                            msl = mask_sb[:, :]
                            nc.vector.tensor_mul(
                                pTb[:, ioff : ioff + 128],
                                pTb[:, ioff : ioff + 128],
                                msl,
                            )
                            nc.vector.tensor_mul(
                                pTb[:, 512 + ioff : 512 + ioff + 128],
                                pTb[:, 512 + ioff : 512 + ioff + 128],
                                msl,
                            )

                    def emit_pv(jt):
                        dpos = jt - 4 * ic
                        ioff = 128 * dpos if dpos > 0 else 0
                        pTb = pTs[jt]
                        pT0 = pTb[:, 0:512]
                        pT1 = pTb[:, 512:1024]
                        last = jt == njt - 1
                        vt = v_sb[jt]
                        nc.tensor.matmul(
                            ps_out[0:64, ioff:512],
                            lhsT=vt[:, pr * 128 : pr * 128 + 64],
                            rhs=pT0[:, ioff:512],
                            start=False,
                            stop=False,
                            tile_position=(0, 0),
                            skip_group_check=True,
                        )
                        nc.tensor.matmul(
                            ps_out[64:128, ioff:512],
                            lhsT=vt[:, pr * 128 + 64 : pr * 128 + 128],
                            rhs=pT1[:, ioff:512],
                            start=False,
                            stop=last,
                            tile_position=(0, 64),
                            skip_group_check=True,
                        )
                        if jt > 0:
                            if ioff == 0:
                                nc.vector.tensor_add(
                                    pacc[:, 0:1024], pacc[:, 0:1024],
                                    pTb[:, 0:1024],
                                )
                            else:
                                nc.vector.tensor_add(
                                    pacc[:, ioff:512],
                                    pacc[:, ioff:512],
                                    pT0[:, ioff:512],
                                )
                                nc.vector.tensor_add(
                                    pacc[:, 512 + ioff : 1024],
                                    pacc[:, 512 + ioff : 1024],
                                    pT1[:, ioff:512],
                                )

                    # PV lags QK/exp by LAG key tiles: its inputs are
                    # always long-ready, so PE drains overlap instead
                    # of waiting on just-finished exps, and the tail
                    # PVs of this pair overlap the next pair's QK/exp
                    # ramp.
                    LAG = 2
                    for jt in range(njt):
                        emit_qk_exp(jt)
                        if jt >= LAG:
                            emit_pv(jt - LAG)
                    for jt in range(max(0, njt - LAG), njt):
                        emit_pv(jt)
                    # Partition-reduce the accumulated p-sums into the
                    # shared denominator bank.  The first write into
                    # each bank (pr 0 / pr 2) clears it via start=True.
                    nc.tensor.matmul(
                        dbank[dp0 : dp0 + 1, :],
                        lhsT=ones_sb,
                        rhs=pacc[:, 0:512],
                        start=False,
                        stop=False,
                        tile_position=(0, dp0),
                        skip_group_check=True,
                    )
                    nc.tensor.matmul(
                        dbank[dp1 : dp1 + 1, :],
                        lhsT=ones_sb,
                        rhs=pacc[:, 512:1024],
                        start=False,
                        stop=False,
                        tile_position=(0, dp1),
                        skip_group_check=True,
                    )
                    # Unnormalized copy frees ps_out quickly (high
                    # priority: it gates the next pair's accumulation
                    # group); normalization happens in-place on att
                    # once the broadcast lands.
                    asl = att[pr][:, isl]
                    with tc.high_priority():
                        nc.vector.tensor_copy(asl, ps_out)
                    if pr % 2 == 1:
                        norm_group(pr // 2)

            def phase_c(s):
                """Output projection for token tiles 4s..4s+4."""
                for tt in range(4 * s, 4 * s + 4):
                    tsl = slice(tt * 128, (tt + 1) * 128)
                    ost = opool.tile([128, 1024], f16, name="ost", tag="ost")
                    for n in range(2):
                        ps = ps_a.tile([128, 512], f32, name="ps_c", tag="psA")
                        for fc in range(4):
                            nc.tensor.matmul(
                                ps,
                                lhsT=att[fc][:, tsl],
                                rhs=wo_sb[fc][:, n * 512 : (n + 1) * 512],
                                start=(fc == 0),
                                stop=(fc == 3),
                            )
                        # ScalarE copy: exp is idle exactly when these
                        # evacuations bunch (stage boundaries), and it
                        # keeps DVE free for the q-chain evacuations
                        # that unblock the next exp stream.
                        nc.scalar.copy(ost[:, n * 512 : (n + 1) * 512], ps)
                    nc.sync.dma_start(out=out[tsl, :], in_=ost)

            for s in range(4):
                phase_a(s)
                if s >= 1:
                    phase_c(s - 1)
                phase_b(s)
            phase_c(3)

    nc.compile()
    return nc


def _get_program():
    if "nc" not in _CACHE:
        _CACHE["nc"] = _build_program()
    return _CACHE["nc"]


def _make_mask():
    # msk[jj, c] = 1 if c >= jj else 0: the 128-wide diagonal block.
    jj = np.arange(128)[:, None]
    c = np.arange(128)[None, :]
    return (c >= jj).astype(np.float16)


def _make_in_maps(x, w_qkv, w_out):
    mask = _make_mask()
    in_maps = []
    for core in range(NCORES):
        b, hg = core // 2, core % 2
        cs = slice(hg * 512, (hg + 1) * 512)
        f16 = np.float16
        in_maps.append(
            {
                "xt": np.ascontiguousarray(x[b].T).astype(f16),
                "wq": np.ascontiguousarray(
                    w_qkv[:, hg * 512 : hg * 512 + 512]
                ).astype(f16),
                "wk": np.ascontiguousarray(
                    w_qkv[:, 1024 + hg * 512 : 1024 + hg * 512 + 512]
                ).astype(f16),
                "wv": np.ascontiguousarray(
                    w_qkv[:, 2048 + hg * 512 : 2048 + hg * 512 + 512]
                ).astype(f16),
                "wo": np.ascontiguousarray(w_out[cs, :]).astype(f16),
                "msk": mask,
            }
        )
    return in_maps


def _run_device(in_maps, trace=False):
    from concourse.bass_utils import run_bass_kernel_spmd

    nc = _get_program()
    return run_bass_kernel_spmd(
        nc, in_maps, core_ids=list(range(NCORES)), trace=trace
    )


def kernel(x, w_qkv, w_out, b_out):
    x = np.asarray(x, dtype=np.float32)
    w_qkv = np.asarray(w_qkv, dtype=np.float32)
    w_out = np.asarray(w_out, dtype=np.float32)
    b_out = np.asarray(b_out, dtype=np.float32)

    res = _run_device(_make_in_maps(x, w_qkv, w_out)).results
    out = np.empty((B, T, C), dtype=np.float32)
    for b in range(B):
        out[b] = (
            res[2 * b]["out"].astype(np.float32)
            + res[2 * b + 1]["out"].astype(np.float32)
            + b_out
        )
    return out
